# revision 1
# baseline (speedup 1.0000x reference)
"""Trainium2 Bass kernel for deterministic NeuralSort soft-kNN (DKNN).

Math (per query b over N neighbors):
    s_j   = -||q_b - x_j||^2
    A_j   = sum_i |s_j - s_i|
    P[r,j]= softmax_j(scaling[r] * s_j - A_j),  r = 0..K-1, scaling[r] = N+1-2(r+1)
    out_j = sum_r P[r,j]

Key reduction: s_j = u_j - ||q_b||^2 with u_j = 2 q_b.x_j - ||x_j||^2.  The
||q||^2 term is constant in j, so it cancels in A (pairwise diffs) and shifts
every softmax row by a constant (scaling[r]*||q||^2) which softmax ignores.
So we never compute ||q||^2.

Sharding: data-parallel over the B=128 queries across 8 cores (16 each);
neighbors replicated.

Per-core hot loop (the O(B_local * N^2) part): for each query b, broadcast
u_b to 128 partitions (DMA), then for each 128-row block of pairwise rows,
one fused op produces |u_j - u_p| with the row-sum accumulated on the fly:
  - ScalarE:  activation(Abs, bias=-u_p, accum_out)      (3 blocks / query)
  - VectorE:  tensor_scalar(add -u_p, abs_max 0, accum)  (5 blocks / query)
By symmetry of |u_j - u_i| the free-dim row sums ARE A_sum for the block's
partition indices, so no cross-partition reduce is needed.
"""

import numpy as np

import concourse.bass as bass
import concourse.bacc as bacc
import concourse.tile as tile
from concourse import mybir
from concourse.masks import make_identity
from concourse.bass_utils import run_bass_kernel_spmd

AFT = mybir.ActivationFunctionType
ALU = mybir.AluOpType
FP32 = mybir.dt.float32
BF16 = mybir.dt.bfloat16

B, N, D, TOPK = 128, 1024, 128, 10
NCORES = 8
BL = B // NCORES          # 16 queries per core
NBLK = N // 128           # 8 row-blocks of the pairwise matrix
GROUPS = 2                # softmax groups (8 queries x 10 rows = 80 partitions)
GB = BL // GROUPS         # 8

# Static engine split of the queries (ratio ~ ACT vs DVE+PE per-query cost).
# ACT queries: fused Abs+accum on ScalarE (self-contained, A in transposed form).
# DVE queries: tensor_scalar |diff| tiles on VectorE, row-reduced by TensorE
# selector-matmuls straight into a row-form PSUM accumulator.
# ACT queries spread across both softmax groups so ScalarE stays busy through
# the whole pairwise phase; groups are {0..7} and {8..15}, group 0's queries
# are scheduled first so its softmax overlaps group 1's pairwise work.
ACT_SET = (0, 1, 2, 8, 9)
G0_DVE = (3, 4, 5, 6, 7)
G1_DVE = (10, 11, 12, 13, 14, 15)
GPS_SET = (10, 12)   # whole queries whose G-pass runs on GpSimd
TAIL_SPLIT = (13, 14, 15)  # final-pair queries: blocks t>=5 go to GpSimd too
PAIRS = ((0, 3), (1, 4), (2, 5), (6, 7), (8, 10), (9, 11), (12, 13), (14, 15))


def _host_consts():
    scaling = (N + 1 - 2.0 * (np.arange(TOPK) + 1)).astype(np.float32)
    E = np.zeros((BL, GROUPS, GB, TOPK), np.float32)
    F = np.zeros((BL, GROUPS, GB, TOPK), np.float32)
    for g in range(GROUPS):
        for bl in range(GB):
            E[g * GB + bl, g, bl, :] = scaling
            F[g * GB + bl, g, bl, :] = -1.0
    G = np.zeros((GB * TOPK, GB), np.float32)
    for bl in range(GB):
        G[bl * TOPK : (bl + 1) * TOPK, bl] = 1.0
    F = F.reshape(BL, -1)
    Fa = F.copy()
    Fd = F.copy()
    for b in range(BL):
        (Fd if b in ACT_SET else Fa)[b, :] = 0.0
    return E.reshape(BL, -1), Fa, Fd, G


def _build_nc(debug_taps=False):
    nc = bacc.Bacc(None, target_bir_lowering=False)

    q_in = nc.dram_tensor("query", [BL, D], FP32, kind="ExternalInput")
    x_in = nc.dram_tensor("neighbors", [N, D], FP32, kind="ExternalInput")
    out_t = nc.dram_tensor("out", [BL, N], FP32, kind="ExternalOutput")
    if debug_taps:
        dbg_u = nc.dram_tensor("dbg_u", [BL, N], FP32, kind="ExternalOutput")
        dbg_a = nc.dram_tensor("dbg_a", [BL, N], FP32, kind="ExternalOutput")
        dbg_nut = nc.dram_tensor("dbg_nut", [128, NBLK * BL], FP32, kind="ExternalOutput")
        dbg_paw = nc.dram_tensor("dbg_paw", [80, N], FP32, kind="ExternalOutput")

    E, Fa, Fd, G = _host_consts()
    e_in = nc.inline_tensor(E, "lhs_e")
    fa_in = nc.inline_tensor(Fa, "lhs_fa")
    fd_in = nc.inline_tensor(Fd, "lhs_fd")
    g_in = nc.inline_tensor(G, "lhs_g")

    with tile.TileContext(nc) as tc:
        with (
            tc.tile_pool(name="consts", bufs=1) as consts,
            tc.tile_pool(name="xp", bufs=1) as xp,
            tc.tile_pool(name="bcast", bufs=4) as bcast,
            tc.tile_pool(name="scrA", bufs=2) as scrA,
            tc.tile_pool(name="scrD", bufs=3) as scrD,
            tc.tile_pool(name="scrP", bufs=2) as scrP,
            tc.tile_pool(name="cmbp", bufs=2) as cmbp,
            tc.tile_pool(name="expp", bufs=2) as expp,
            tc.tile_pool(name="small", bufs=8) as small,
            tc.tile_pool(name="dramp", bufs=1, space="DRAM") as dramp,
        ):
            ident = consts.tile([128, 128], FP32)
            make_identity(nc, ident)
            ones128 = consts.tile([128, 1], FP32)
            nc.vector.memset(ones128, 1.0)
            ones1xb = consts.tile([1, BL], FP32)
            nc.vector.memset(ones1xb, 1.0)

            # ---- Phase A: neighbors in, transpose to [d, j]; row norms ----
            x_sb = xp.tile([128, NBLK, D], FP32)
            xv = x_in[:].rearrange("(t p) d -> p t d", p=128)
            half = NBLK // 2
            nc.default_dma_engine.dma_start(out=x_sb[:, :half, :], in_=xv[:, :half, :])
            q_sb = small.tile([BL, D], FP32)
            nc.default_dma_engine.dma_start(out=q_sb, in_=q_in[:])
            nc.default_dma_engine.dma_start(out=x_sb[:, half:, :], in_=xv[:, half:, :])

            xT = xp.tile([128, N], FP32)  # xT[d, j] = X[j, d]
            with tc.tile_pool(name="ps_tr", bufs=2, space="PSUM") as ps_tr:
                for t in range(NBLK):
                    ptr = ps_tr.tile([128, 128], FP32)
                    nc.tensor.transpose(ptr, x_sb[:, t, :], ident)
                    nc.any.tensor_copy(xT[:, t * 128 : (t + 1) * 128], ptr)

            sq = xp.tile([128, N], FP32)
            negx2 = consts.tile([1, N], FP32)  # -||x_j||^2
            with tc.tile_pool(name="ps_x2", bufs=1, space="PSUM") as ps_x2:
                px2 = ps_x2.tile([1, N], FP32)
                for c in range(2):
                    cs = slice(c * 512, (c + 1) * 512)
                    nc.scalar.activation(out=sq[:, cs], in_=xT[:, cs], func=AFT.Square)
                    nc.tensor.matmul(
                        px2[:, cs], lhsT=ones128, rhs=sq[:, cs], start=True, stop=True
                    )
                    nc.scalar.activation(
                        out=negx2[:, cs], in_=px2[:, cs], func=AFT.Copy, scale=-1.0
                    )

            e_sb = consts.tile([BL, GROUPS * GB * TOPK], FP32)
            nc.default_dma_engine.dma_start(out=e_sb, in_=e_in[:])
            fa_sb = consts.tile([BL, GROUPS * GB * TOPK], FP32)
            nc.default_dma_engine.dma_start(out=fa_sb, in_=fa_in[:])
            fd_sb = consts.tile([BL, GROUPS * GB * TOPK], FP32)
            nc.default_dma_engine.dma_start(out=fd_sb, in_=fd_in[:])
            g_sb = consts.tile([GB * TOPK, GB], FP32)
            nc.default_dma_engine.dma_start(out=g_sb, in_=g_in[:])

            # ---- Phase B: u = 2 Q X^T - ||x||^2, plus -u^T columns ----
            q2T = consts.tile([128, BL], FP32)   # (2Q)^T
            u_sb = consts.tile([BL, N], FP32)
            nuT = consts.tile([128, NBLK, BL], FP32)  # nuT[p, t, b] = -u[b, t*128+p]
            u_dram = dramp.tile([BL, N], FP32)
            with tc.tile_pool(name="ps_qt", bufs=2, space="PSUM") as ps_qt:
                pqt = ps_qt.tile([128, BL], FP32)
                nc.tensor.transpose(pqt, q_sb, ident[:BL, :BL])
                nc.scalar.activation(out=q2T, in_=pqt, func=AFT.Copy, scale=2.0)
                # nuT[:, t, b] = -u[b, t*128+p] computed directly:
                # uT_blk = xT_blk^T @ q2T + negx2_blk^T @ ones  (bit-identical
                # to the u_sb path: same products, same accumulation order).
                for t in range(NBLK):
                    put = ps_qt.tile([128, BL], FP32, tag="put")
                    nc.tensor.matmul(
                        put, lhsT=xT[:, t * 128 : (t + 1) * 128], rhs=q2T,
                        start=True, stop=False,
                    )
                    nc.tensor.matmul(
                        put, lhsT=negx2[:, t * 128 : (t + 1) * 128], rhs=ones1xb,
                        start=False, stop=True,
                    )
                    nc.vector.tensor_scalar(
                        out=nuT[:, t, :], in0=put, scalar1=-1.0, scalar2=None,
                        op0=ALU.mult,
                    )
                with tc.tile_pool(name="ps_u", bufs=1, space="PSUM") as ps_u:
                    pu = ps_u.tile([BL, N], FP32)
                    for c in range(2):
                        cs = slice(c * 512, (c + 1) * 512)
                        nc.tensor.matmul(
                            pu[:, cs], lhsT=q2T, rhs=xT[:, cs], start=True, stop=False
                        )
                        nc.tensor.matmul(
                            pu[:, cs], lhsT=ones1xb, rhs=negx2[:, cs],
                            start=False, stop=True,
                        )
                    nc.any.tensor_copy(u_sb, pu)
            nc.default_dma_engine.dma_start(out=u_dram[:4, :], in_=u_sb[:4, :])
            nc.scalar.dma_start(out=u_dram[4:, :], in_=u_sb[4:, :])
            Tb = small.tile([BL, 1], FP32, tag="Tb")
            nc.vector.tensor_reduce(
                out=Tb, in_=u_sb, axis=mybir.AxisListType.X, op=ALU.add
            )

            # Three-limb bf16 split of -nuT = u^T: u = hi + mid + lo exactly
            # (3 x 8 mantissa bits cover fp32's 24).  The 0/1 compare matrix G
            # is exact in bf16, so the TensorE reduce of (counts, L_hi, L_mid,
            # L_lo) runs at bf16 rate (1 col/cycle) instead of fp32's 4.
            hi_bf = consts.tile([128, NBLK, BL], BF16)
            nc.vector.tensor_scalar(out=hi_bf[:].rearrange("p t b -> p (t b)"),
                                    in0=nuT[:].rearrange("p t b -> p (t b)"),
                                    scalar1=-1.0, scalar2=None, op0=ALU.mult)
            hi32 = consts.tile([128, NBLK * BL], FP32)
            nc.vector.tensor_copy(hi32, hi_bf[:].rearrange("p t b -> p (t b)"))
            r1 = consts.tile([128, NBLK * BL], FP32)  # = hi - u
            nc.vector.tensor_tensor(out=r1, in0=nuT[:].rearrange("p t b -> p (t b)"),
                                    in1=hi32, op=ALU.add)
            mid_bf = consts.tile([128, NBLK, BL], BF16)
            nc.vector.tensor_scalar(out=mid_bf[:].rearrange("p t b -> p (t b)"),
                                    in0=r1, scalar1=-1.0, scalar2=None, op0=ALU.mult)
            mid32 = consts.tile([128, NBLK * BL], FP32)
            nc.vector.tensor_copy(mid32, mid_bf[:].rearrange("p t b -> p (t b)"))
            r2 = consts.tile([128, NBLK * BL], FP32)  # = hi + mid - u
            nc.vector.tensor_tensor(out=r2, in0=r1, in1=mid32, op=ALU.add)
            lo_bf = consts.tile([128, NBLK, BL], BF16)
            nc.vector.tensor_scalar(out=lo_bf[:].rearrange("p t b -> p (t b)"),
                                    in0=r2, scalar1=-1.0, scalar2=None, op0=ALU.mult)

            # W[p, t, b, m] (bf16): lhsT for the TensorE reduce of G.
            # col m==b: 1.0 -> counts r_j; m==32+b: hi -> L_hi row 32+b;
            # m==48+b: mid -> L_mid row 48+b; m==64+b: lo -> L_lo row 64+b.
            MW = 80
            W = consts.tile([128, NBLK, BL, MW], BF16)
            zb = consts.tile([128, 1], BF16)
            nc.vector.memset(zb, 0.0)
            zv = zb[:]
            zap = bass.AP(tensor=zv.tensor, offset=zv.offset,
                          ap=[zv.ap[0], [0, NBLK * BL * MW]])
            nc.gpsimd.affine_select(
                out=W[:].rearrange("p t b m -> p (t b m)"), in_=zap,
                compare_op=ALU.not_equal, fill=1.0, base=0,
                pattern=[[0, NBLK], [1, BL], [-1, MW]], channel_multiplier=0,
            )
            for t in range(NBLK):
                for lane, limb in ((32, hi_bf), (48, mid_bf), (64, lo_bf)):
                    lv = limb[:, t, :]
                    lb = bass.AP(tensor=lv.tensor, offset=lv.offset,
                                 ap=[lv.ap[0], list(lv.ap[1]), [0, BL]])
                    nc.gpsimd.tensor_tensor(
                        out=W[:, t, :, lane : lane + BL],
                        in0=W[:, t, :, 0:BL], in1=lb, op=ALU.mult,
                    )

            # ---- Phases C/D/E: pairwise passes, per-group overlap ----
            asumT = consts.tile([128, BL, NBLK], FP32)  # A_sum[b][t*128+p], ACT qs
            Ag0 = consts.tile([BL, N], FP32)
            Ag1 = consts.tile([BL, N], FP32)
            nc.gpsimd.memset(Ag0[:], 0.0)
            nc.gpsimd.memset(Ag1[:], 0.0)

            pm_cm = tc.tile_pool(name="ps_pm", bufs=1, space="PSUM")
            ps_pm = pm_cm.__enter__()
            po_cm = tc.tile_pool(name="ps_out", bufs=1, space="PSUM")
            ps_out = po_cm.__enter__()
            pm_tile = {}

            def phase_e_prelude(g):
                # pm = E . u can run as soon as u_sb is ready; F . A joins later
                pm = ps_pm.tile([GB * TOPK, N], FP32, tag="pm", name=f"pm{g}")
                pm_tile[g] = pm
                for c in range(2):
                    cs = slice(c * 512, (c + 1) * 512)
                    nc.tensor.matmul(
                        pm[:, cs], lhsT=e_sb[:, g * 80 : (g + 1) * 80],
                        rhs=u_sb[:, cs], start=True, stop=False,
                    )

            pa_cm, pa_tile, first = {}, {}, {}
            remaining = {0: len(G0_DVE) * NBLK, 1: len(G1_DVE) * NBLK}
            for g in (1, 0):  # stack allocator: group 0's pool closes first
                pa_cm[g] = tc.tile_pool(name=f"ps_pa{g}", bufs=1, space="PSUM")
                pool = pa_cm[g].__enter__()
                pa_tile[g] = pool.tile([MW, N], FP32, tag=f"pa{g}", name=f"pa{g}")
                first[g] = [True, True]

            def emit_act_query(b, ub):
                for t in range(NBLK):
                    sa = scrA.tile([128, N], FP32, tag="sa")
                    nc.scalar.activation(
                        out=sa, in_=ub, func=AFT.Abs,
                        bias=nuT[:, t, b : b + 1], scale=1.0,
                        accum_out=asumT[:, b, t : t + 1],
                    )

            def emit_dve_query(b, ub):
                g = 0 if b < GB else 1
                pa = pa_tile[g]
                for t in range(NBLK):
                    gps = b in GPS_SET or (b in TAIL_SPLIT and t >= 5)
                    eng = nc.gpsimd if gps else nc.vector
                    pool = scrP if gps else scrD
                    sd = pool.tile([128, N], BF16, tag="sdp" if gps else "sd", name="sd")
                    eng.tensor_scalar(
                        out=sd, in0=ub, scalar1=nuT[:, t, b : b + 1], scalar2=0.0,
                        op0=ALU.add, op1=ALU.is_gt,
                    )
                    remaining[g] -= 1
                    for c in range(2):
                        cs = slice(c * 512, (c + 1) * 512)
                        nc.tensor.matmul(
                            pa[:, cs], lhsT=W[:, t, b], rhs=sd[:, cs],
                            start=first[g][c], stop=remaining[g] == 0,
                        )
                        first[g][c] = False

            def combine_half(g, paw_g):
                # A[b,j] = u*(2r - N) + (T_b - 2(L_hi+L_mid+L_lo)), all 16 rows
                # (rows not hosted in this half read accumulated zeros -> junk,
                # only the hosted rows are consumed).  Group 0 (overlapped, not
                # latency-critical) sums the limb rows with accumulating SWDGE
                # DMAs; group 1 (the tail) uses three parallel DMAs on separate
                # issuers plus DVE adds.
                Lsum = consts.tile([BL, N], FP32, tag="Lsum")
                Lhi = consts.tile([BL, N], FP32, tag="Lhi")
                nc.sync.dma_start(out=Lhi, in_=paw_g[32 : 32 + BL, :])
                Lmid = consts.tile([BL, N], FP32, tag="Lmid")
                nc.scalar.dma_start(out=Lmid, in_=paw_g[48 : 48 + BL, :])
                Llo = consts.tile([BL, N], FP32, tag="Llo")
                nc.gpsimd.dma_start(out=Llo, in_=paw_g[64 : 64 + BL, :])
                tLa = cmbp.tile([BL, N], FP32, tag="cmbLa")
                nc.vector.tensor_tensor(out=tLa, in0=Lhi, in1=Lmid, op=ALU.add)
                nc.vector.tensor_tensor(out=Lsum, in0=tLa, in1=Llo, op=ALU.add)
                t1 = cmbp.tile([BL, N], FP32, tag="cmb1")
                nc.vector.tensor_scalar(
                    out=t1, in0=pa_tile[g][:BL, :], scalar1=2.0, scalar2=-float(N),
                    op0=ALU.mult, op1=ALU.add,
                )
                t2 = cmbp.tile([BL, N], FP32, tag="cmb2")
                nc.vector.tensor_tensor(out=t2, in0=t1, in1=u_sb, op=ALU.mult)
                t3 = cmbp.tile([BL, N], FP32, tag="cmb3")
                nc.vector.tensor_scalar(
                    out=t3, in0=Lsum, scalar1=-2.0, scalar2=Tb,
                    op0=ALU.mult, op1=ALU.add,
                )
                cmb = consts.tile([BL, N], FP32, tag=f"cmb{g}")
                nc.vector.tensor_tensor(out=cmb, in0=t2, in1=t3, op=ALU.add)
                return cmb

            def act_rows_to_ag(g, b0, nq, Ag):
                # DMA-transpose asumT[:, b0:b0+nq, :] into row form via a DRAM
                # bounce (src contiguity is 8-element runs along t -> cheap).
                adr = dramp.tile([nq, N], FP32, tag=f"adr{g}", name=f"adr{g}")
                dst = bass.AP(
                    tensor=adr[:].tensor, offset=adr[:].offset,
                    ap=[[1, 128], [N, nq], [128, NBLK]],
                )
                nc.sync.dma_start(out=dst, in_=asumT[:, b0 : b0 + nq, :])
                nc.scalar.dma_start(out=Ag[b0 : b0 + nq, :], in_=adr[:])

            def phase_e_act_part(g, Ag):
                # F.A contribution of the ACT-path rows; can run as soon as the
                # transposed ACT A-rows land, well before the DVE combine.
                pm = pm_tile[g]
                for c in range(2):
                    cs = slice(c * 512, (c + 1) * 512)
                    nc.tensor.matmul(
                        pm[:, cs], lhsT=fa_sb[:, g * 80 : (g + 1) * 80],
                        rhs=Ag[:, cs], start=False, stop=False,
                    )

            def phase_e(g, cmb):
                pm = pm_tile[g]
                for c in range(2):
                    cs = slice(c * 512, (c + 1) * 512)
                    nc.tensor.matmul(
                        pm[:, cs], lhsT=fd_sb[:, g * 80 : (g + 1) * 80],
                        rhs=cmb[:, cs], start=False, stop=True,
                    )
                nmx = small.tile([GB * TOPK, 1], FP32, tag="nmx")
                nc.vector.tensor_reduce(
                    out=nmx, in_=pm, axis=mybir.AxisListType.X,
                    op=ALU.max, negate=True,
                )
                exps = expp.tile([GB * TOPK, N], BF16, tag="exps")
                den = small.tile([GB * TOPK, 1], FP32, tag="den")
                nc.scalar.activation(
                    out=exps, in_=pm, func=AFT.Exp, bias=nmx, scale=1.0,
                    accum_out=den,
                )
                rden = small.tile([GB * TOPK, 1], FP32, tag="rden")
                nc.vector.reciprocal(rden, den)
                gr = small.tile([GB * TOPK, GB], BF16, tag="gr")
                nc.vector.tensor_scalar(
                    out=gr, in0=g_sb, scalar1=rden, scalar2=None, op0=ALU.mult
                )
                po = ps_out.tile([GB, N], FP32, tag="po", name=f"po{g}")
                og = expp.tile([GB, N], FP32, tag="og")
                for c in range(2):
                    cs = slice(c * 512, (c + 1) * 512)
                    nc.tensor.matmul(
                        po[:, cs], lhsT=gr, rhs=exps[:, cs], start=True, stop=True
                    )
                    nc.any.tensor_copy(og[:, cs], po[:, cs])
                    eng = nc.sync if c == 0 else nc.scalar
                    eng.dma_start(
                        out=out_t[g * GB : (g + 1) * GB, cs], in_=og[:, cs]
                    )

            def finalize_group(g):
                paw_g = consts.tile([MW, N], FP32, tag=f"paw{g}")
                nc.any.tensor_copy(paw_g, pa_tile[g])
                cmb = combine_half(g, paw_g)
                pa_cm[g].__exit__(None, None, None)
                phase_e(g, cmb)
                return paw_g

            paw0 = None
            for pi, (b0, b1) in enumerate(PAIRS):
                ub2 = bcast.tile([128, 2, N], FP32)
                base = u_dram[b0 : b0 + 1, :]
                src = bass.AP(
                    tensor=base.tensor, offset=base.offset,
                    ap=[[0, 128], [(b1 - b0) * N, 2], [1, N]],
                )
                nc.default_dma_engine.dma_start(out=ub2, in_=src)
                for k, b in enumerate((b0, b1)):
                    ub = ub2[:, k, :]
                    if b in ACT_SET:
                        emit_act_query(b, ub)
                    else:
                        emit_dve_query(b, ub)
                if pi == 2:
                    phase_e_prelude(0)
                    act_rows_to_ag(0, 0, 3, Ag0)
                    phase_e_act_part(0, Ag0)
                if pi == 3:
                    paw0 = finalize_group(0)
                if pi == 5:
                    phase_e_prelude(1)
                    act_rows_to_ag(1, 8, 2, Ag1)
                    phase_e_act_part(1, Ag1)
            finalize_group(1)
            po_cm.__exit__(None, None, None)
            pm_cm.__exit__(None, None, None)

            if debug_taps:
                nc.default_dma_engine.dma_start(out=dbg_u[:], in_=u_sb)
                nc.default_dma_engine.dma_start(out=dbg_a[:8], in_=Ag0[:8, :])
                nc.default_dma_engine.dma_start(out=dbg_a[8:], in_=Ag1[8:, :])
                nc.default_dma_engine.dma_start(
                    out=dbg_nut[:], in_=nuT[:].rearrange("p t b -> p (t b)")
                )
                nc.default_dma_engine.dma_start(out=dbg_paw[:], in_=paw0)

    nc.compile()
    return nc


_CACHE = {}


def _get_nc():
    if "nc" not in _CACHE:
        _CACHE["nc"] = _build_nc()
    return _CACHE["nc"]


def _in_maps(query, neighbors):
    query = np.ascontiguousarray(query, dtype=np.float32)
    neighbors = np.ascontiguousarray(neighbors, dtype=np.float32)
    return [
        {"query": query[c * BL : (c + 1) * BL], "neighbors": neighbors}
        for c in range(NCORES)
    ]


def _run(query, neighbors, **kw):
    nc = _get_nc()
    res = run_bass_kernel_spmd(nc, _in_maps(query, neighbors), list(range(NCORES)), **kw)
    out = np.concatenate([res.results[c]["out"] for c in range(NCORES)], axis=0)
    return out, res


def kernel(query, neighbors):
    out, _ = _run(query, neighbors)
    return out


def run_profiled(query, neighbors, **kw):
    out, res = _run(query, neighbors, trace=True, **kw)
    return out, res

